# revision 3
# baseline (speedup 1.0000x reference)
"""ConvCapsule dynamic-routing kernel for 8 Trainium2 NeuronCores.

Sharding: pure data parallel over batch B=256 -> 32 examples per core.
W replicated. Routing state is per-example so no cross-core comm.

Device layout (per core, B=32 local):
  partitions p = (t, b), t = ijk%4 in {0..3}, b in {0..31}  -> 128 partitions
  U (u_hat, bf16):  [128, G*O*E] with free = (g, o, e), g = ijk//4 in {0..287}
  bb (logits fp32): [128, G*O]   free = (g, o)

u_hat build: per group g of 4 positions, block-diag stationary
  xbd[g] [K=32=(t,d), M=128=(t,b)], moving W_stk[g] [32, 160=(o,e)]
  -> psum [128, 160] = u_hat for those 4 positions, all 32 b.

Iteration 0: c = softmax(0) = 0.1 exactly -> s0 = 0.1 * sum_ijk u_hat
  computed directly as one K=9216 PSUM-accumulated matmul chain.

Iterations: db = sum_e U*v (DVE mul + segmented reduce), softmax on
  ACT/DVE, s = sum_g c*U (DVE mul + strided reduce) + t-fold matmul.
"""

import sys

import numpy as np

sys.path.insert(0, "/opt/trn_rl_repo")

import ml_dtypes  # noqa: E402

BF16 = ml_dtypes.bfloat16

# Problem constants (hardcoded per contract)
B_FULL = 256
NCORES = 8
B = B_FULL // NCORES  # 32 per core
IJK = 32 * 6 * 6  # 1152
O, E, D = 10, 16, 8
OE = O * E  # 160
T = 4  # positions per u_hat matmul group
G = IJK // T  # 288 groups
CH = 3  # routing chunks over g
GC = G // CH  # 96 groups per chunk
KC = 72  # K-chunks for the s0 matmul (16 pos x 8 d = 128 each)
ITERS = 3

_CACHE: dict = {}


def _build_program():
    from contextlib import ExitStack

    from concourse import bacc, mybir, tile

    nc = bacc.Bacc(
        "TRN2", target_bir_lowering=False, debug=False, num_devices=NCORES
    )
    dt = mybir.dt
    AF = mybir.ActivationFunctionType
    ALU = mybir.AluOpType
    AX = mybir.AxisListType

    # ---- DRAM I/O ----
    xbd_d = nc.dram_tensor("xbd", [32, G * 128], dt.bfloat16, kind="ExternalInput").ap()
    wstk_d = nc.dram_tensor("wstk", [32, G * OE], dt.bfloat16, kind="ExternalInput").ap()
    xalt_d = nc.dram_tensor("xalt", [128, KC * B], dt.bfloat16, kind="ExternalInput").ap()
    walt_d = nc.dram_tensor("walt", [128, KC * OE], dt.bfloat16, kind="ExternalInput").ap()
    fold_d = nc.dram_tensor("fold", [128, B], dt.float32, kind="ExternalInput").ap()
    bcast_d = nc.dram_tensor("bcast", [32, 128], dt.float32, kind="ExternalInput").ap()
    vout_d = nc.dram_tensor("v_out", [B, OE], dt.float32, kind="ExternalOutput").ap()

    with tile.TileContext(nc) as tc, ExitStack() as ctx:
        per = ctx.enter_context(tc.tile_pool(name="per", bufs=1))
        # Persistent tiles
        U = per.tile([128, G * OE], dt.bfloat16, tag="U")
        bb = per.tile([128, G * O], dt.float32, tag="bb")
        fold_t = per.tile([128, B], dt.float32, tag="fold")
        bcast_t = per.tile([32, 128], dt.float32, tag="bcast")
        nc.sync.dma_start(fold_t[:], fold_d[:])
        nc.sync.dma_start(bcast_t[:], bcast_d[:])

        # ---------- Phase 1a: s0 via fully-contracted matmul ----------
        with tc.tile_pool(name="s0pool", bufs=4) as s0p, tc.tile_pool(
            name="ps0pool", bufs=1, space="PSUM"
        ) as ps0p:
            ps0 = ps0p.tile([B, OE], dt.float32, tag="ps0")
            for k in range(KC):
                xa = s0p.tile([128, B], dt.bfloat16, tag="xa")
                wa = s0p.tile([128, OE], dt.bfloat16, tag="wa")
                nc.sync.dma_start(xa[:], xalt_d[:, k * B : (k + 1) * B])
                nc.sync.dma_start(wa[:], walt_d[:, k * OE : (k + 1) * OE])
                nc.tensor.matmul(
                    ps0[:], xa[:], wa[:], start=(k == 0), stop=(k == KC - 1)
                )
            s0raw = per.tile([B, OE], dt.float32, tag="s0raw")
            nc.vector.tensor_copy(s0raw[:], ps0[:])

        # ---------- Phase 1b: u_hat ----------
        DG = 24  # groups per DMA chunk
        with tc.tile_pool(name="uin", bufs=3) as uin, tc.tile_pool(
            name="upsum", bufs=4, space="PSUM"
        ) as ups:
            for c0 in range(G // DG):
                xb = uin.tile([32, DG * 128], dt.bfloat16, tag="xb")
                wk = uin.tile([32, DG * OE], dt.bfloat16, tag="wk")
                nc.sync.dma_start(xb[:], xbd_d[:, c0 * DG * 128 : (c0 + 1) * DG * 128])
                nc.sync.dma_start(wk[:], wstk_d[:, c0 * DG * OE : (c0 + 1) * DG * OE])
                for j in range(DG):
                    g = c0 * DG + j
                    psu = ups.tile([128, OE], dt.float32, tag="psu")
                    nc.tensor.matmul(
                        psu[:],
                        xb[:, j * 128 : (j + 1) * 128],
                        wk[:, j * OE : (j + 1) * OE],
                        start=True,
                        stop=True,
                    )
                    eng = nc.vector if (g % 2 == 0) else nc.scalar
                    if eng is nc.vector:
                        nc.vector.tensor_copy(U[:, g * OE : (g + 1) * OE], psu[:])
                    else:
                        nc.scalar.copy(U[:, g * OE : (g + 1) * OE], psu[:])

        # ---------- Helpers ----------
        def squash_and_v(s_raw_ap, alpha, vbf_tile, final_out=None):
            """s = alpha * s_raw; v = s*sn/(1+sn)/(sqrt(sn)+1e-6).
            Writes bf16 v (if vbf_tile) and/or fp32 final output."""
            sq = per.tile([B, OE], dt.float32, tag="sq")
            nc.vector.tensor_mul(sq[:], s_raw_ap, s_raw_ap)
            snr = per.tile([B, O], dt.float32, tag="snr")
            nc.vector.reduce_sum(
                snr[:].unsqueeze(2),
                sq[:].rearrange("p (o e) -> p o e", e=E),
                axis=AX.X,
            )
            sn = per.tile([B, O], dt.float32, tag="sn")
            nc.scalar.mul(sn[:], snr[:], alpha * alpha)
            d1 = per.tile([B, O], dt.float32, tag="d1")
            nc.scalar.add(d1[:], sn[:], 1.0)
            r1 = per.tile([B, O], dt.float32, tag="r1")
            nc.vector.reciprocal(r1[:], d1[:])
            rt = per.tile([B, O], dt.float32, tag="rt")
            nc.scalar.sqrt(rt[:], sn[:])
            nc.vector.tensor_scalar_add(rt[:], rt[:], 1e-6)
            r2 = per.tile([B, O], dt.float32, tag="r2")
            nc.vector.reciprocal(r2[:], rt[:])
            fac = per.tile([B, O], dt.float32, tag="fac")
            nc.vector.tensor_mul(fac[:], sn[:], r1[:])
            nc.vector.tensor_mul(fac[:], fac[:], r2[:])
            nc.scalar.mul(fac[:], fac[:], alpha)
            fac_b = fac[:].unsqueeze(2).broadcast_to([B, O, E])
            v32 = per.tile([B, OE], dt.float32, tag="v32")
            nc.vector.tensor_tensor(
                v32[:].rearrange("p (o e) -> p o e", e=E),
                s_raw_ap.rearrange("p (o e) -> p o e", e=E),
                fac_b,
                op=ALU.mult,
            )
            if final_out is not None:
                nc.sync.dma_start(final_out, v32[:])
                return None
            # broadcast v32 [32,160] -> vbc [128,160] bf16 via matmul
            with tc.tile_pool(name="psb", bufs=1, space="PSUM") as psbp:
                psb = psbp.tile([128, OE], dt.float32, tag="psb")
                nc.tensor.matmul(psb[:], bcast_t[:], v32[:], start=True, stop=True)
                nc.vector.tensor_copy(vbf_tile[:], psb[:])
            return None

        def db_pass(vbc_tile, first):
            """bb (+)= sum_e U * vbc ; one chunk at a time."""
            for c in range(CH):
                prod = per.tile([128, GC * OE], dt.bfloat16, tag="prod")
                vb = (
                    vbc_tile[:]
                    .rearrange("p (o e) -> p o e", e=E)
                    .unsqueeze(1)
                    .broadcast_to([128, GC, O, E])
                )
                nc.vector.tensor_tensor(
                    prod[:].rearrange("p (g o e) -> p g o e", o=O, e=E),
                    U[:, c * GC * OE : (c + 1) * GC * OE].rearrange(
                        "p (g o e) -> p g o e", o=O, e=E
                    ),
                    vb,
                    op=ALU.mult,
                )
                dst = bb[:, c * GC * O : (c + 1) * GC * O]
                if first:
                    nc.vector.reduce_sum(
                        dst.unsqueeze(2),
                        prod[:].rearrange("p (go e) -> p go e", e=E),
                        axis=AX.X,
                    )
                else:
                    tmp = per.tile([128, GC * O], dt.float32, tag="dbtmp")
                    nc.vector.reduce_sum(
                        tmp[:].unsqueeze(2),
                        prod[:].rearrange("p (go e) -> p go e", e=E),
                        axis=AX.X,
                    )
                    nc.vector.tensor_add(dst, dst, tmp[:])

        def softmax_c(c_tile):
            """c = softmax(bb) over o -> bf16 [128, G*O]."""
            ex = per.tile([128, G * O], dt.bfloat16, tag="ex")
            nc.scalar.activation(ex[:], bb[:], AF.Exp)
            den = per.tile([128, G], dt.float32, tag="den")
            nc.vector.reduce_sum(
                den[:].unsqueeze(2),
                ex[:].rearrange("p (g o) -> p g o", o=O),
                axis=AX.X,
            )
            rec = per.tile([128, G], dt.float32, tag="rec")
            nc.vector.reciprocal(rec[:], den[:])
            rec_b = rec[:].unsqueeze(2).broadcast_to([128, G, O])
            nc.vector.tensor_tensor(
                c_tile[:].rearrange("p (g o) -> p g o", o=O),
                ex[:].rearrange("p (g o) -> p g o", o=O),
                rec_b,
                op=ALU.mult,
            )

        def s_pass(c_tile, alpha, vbf_tile, final_out=None):
            """s_raw32 = sum_{g,t} c*U (fold via matmul); then squash."""
            rs = per.tile([128, OE], dt.float32, tag="rs")
            for c in range(CH):
                prod = per.tile([128, GC * OE], dt.bfloat16, tag="prod")
                cb = (
                    c_tile[:, c * GC * O : (c + 1) * GC * O]
                    .rearrange("p (g o) -> p g o", o=O)
                    .unsqueeze(3)
                    .broadcast_to([128, GC, O, E])
                )
                nc.vector.tensor_tensor(
                    prod[:].rearrange("p (g o e) -> p g o e", o=O, e=E),
                    U[:, c * GC * OE : (c + 1) * GC * OE].rearrange(
                        "p (g o e) -> p g o e", o=O, e=E
                    ),
                    cb,
                    op=ALU.mult,
                )
                # reduce over g (strided innermost)
                rst = per.tile([128, OE], dt.float32, tag="rst")
                nc.vector.reduce_sum(
                    rst[:].unsqueeze(2),
                    prod[:].rearrange("p (g oe) -> p oe g", oe=OE),
                    axis=AX.X,
                )
                if c == 0:
                    nc.vector.tensor_copy(rs[:], rst[:])
                else:
                    nc.vector.tensor_add(rs[:], rs[:], rst[:])
            with tc.tile_pool(name="pss", bufs=1, space="PSUM") as pssp:
                pss = pssp.tile([B, OE], dt.float32, tag="pss")
                nc.tensor.matmul(pss[:], fold_t[:], rs[:], start=True, stop=True)
                sfold = per.tile([B, OE], dt.float32, tag="sfold")
                nc.vector.tensor_copy(sfold[:], pss[:])
            squash_and_v(sfold[:], alpha, vbf_tile, final_out=final_out)

        # ---------- Iterations ----------
        vbf = per.tile([128, OE], dt.bfloat16, tag="vbf")
        cbf = per.tile([128, G * O], dt.bfloat16, tag="cbf")

        # iter 0: c uniform -> v0 from s0raw * 0.1
        squash_and_v(s0raw[:], 1.0 / O, vbf)
        # iter 1
        db_pass(vbf, first=True)
        softmax_c(cbf)
        s_pass(cbf, 1.0, vbf)
        # iter 2 (final)
        db_pass(vbf, first=False)
        softmax_c(cbf)
        s_pass(cbf, 1.0, vbf, final_out=vout_d[:])

    nc.compile()
    return nc


def _host_prep(x: np.ndarray, W: np.ndarray):
    """Build per-core input maps (host-side layout prep, all numpy)."""
    x = np.ascontiguousarray(x.reshape(B_FULL, IJK, D))
    w = np.ascontiguousarray(W.reshape(IJK, O, D, E))

    # W-derived arrays (identical across cores)
    # wstk: [32=(t,d), G*OE] ; wstk[(t,d), g*OE + o*16+e] = w[4g+t, o, d, e]
    wg = w.reshape(G, T, O, D, E)  # [g,t,o,d,e]
    wstk = np.ascontiguousarray(wg.transpose(1, 3, 0, 2, 4)).reshape(T * D, G * OE)
    wstk = wstk.astype(BF16)
    # walt: [128=(j,d), KC*OE] ; walt[(j,d), k*OE + o*16+e] = w[16k+j, o, d, e]
    wk = w.reshape(KC, 16, O, D, E)  # [k,j,o,d,e]
    walt = np.ascontiguousarray(wk.transpose(1, 3, 0, 2, 4)).reshape(128, KC * OE)
    walt = walt.astype(BF16)
    # fold: [128=(t,b), B] one where b matches
    fold = np.zeros((128, B), np.float32)
    for t in range(T):
        fold[t * B : (t + 1) * B, :] = np.eye(B, dtype=np.float32)
    # bcast: [32=b, 128=(t,b')] delta
    bcast = np.ascontiguousarray(fold.T)

    in_maps = []
    for core in range(NCORES):
        xc = x[core * B : (core + 1) * B]  # [32, IJK, 8]
        xg = xc.reshape(B, G, T, D)  # [b,g,t,d]
        # xbd: [32=(t,d) rows, G*128 cols=(g,(t',b))] block diag
        xbd = np.zeros((T, D, G, T, B), np.float32)
        for t in range(T):
            xbd[t, :, :, t, :] = xg[:, :, t, :].transpose(2, 1, 0)  # [d,g,b]
        xbd = xbd.transpose(0, 1, 2, 3, 4).reshape(T * D, G * T * B).astype(BF16)
        # xalt: [128=(j,d), KC*B] ; xalt[(j,d), k*B+b] = xc[b, 16k+j, d]
        xk = xc.reshape(B, KC, 16, D)  # [b,k,j,d]
        xalt = np.ascontiguousarray(xk.transpose(2, 3, 1, 0)).reshape(128, KC * B)
        xalt = xalt.astype(BF16)
        in_maps.append(
            {
                "xbd": np.ascontiguousarray(xbd),
                "wstk": wstk,
                "xalt": xalt,
                "walt": walt,
                "fold": fold,
                "bcast": bcast,
            }
        )
    return in_maps


def kernel(x: np.ndarray, W: np.ndarray, _trace: bool = False):
    from concourse.bass_utils import run_bass_kernel_spmd

    if "nc" not in _CACHE:
        _CACHE["nc"] = _build_program()
    nc = _CACHE["nc"]
    in_maps = _host_prep(np.asarray(x, np.float32), np.asarray(W, np.float32))
    res = run_bass_kernel_spmd(
        nc, in_maps, core_ids=list(range(NCORES)), trace=_trace
    )
    _CACHE["last_result"] = res
    out = np.stack([r["v_out"] for r in res.results])  # [8, 32, 160]
    return out.reshape(B_FULL, O, E).astype(np.float32)
